# revision 1
# baseline (speedup 1.0000x reference)
"""Self-contained Trainium2 Bass kernel for the differentiable A* forward pass.

Contract: kernel(**inputs) takes the FULL unsharded inputs (start_index,
goal_index, cost_maps, nodes, adj, weighted_adj) and returns the full output
(histories, path_maps), matching reference() exactly.

Strategy: the 1024-step t-loop is inherently serial, so the whole search runs
on one NeuronCore and the identical kernel is replicated across all 8 cores
(inputs replicated; core 0's output is used). Per step the kernel does a
two-level argmax over the frontier value vector (4096 nodes as [64,64] SBUF
tiles), an indirect-DMA gather of weighted_adj[ind] (64 chunks x 256B), and
one-hot masked state updates -- all register-free (this toolchain's
sequencer SBUF loads are broken on HW). The frontier value `val` is
maintained incrementally and state updates are software-pipelined into the
next step's PE/DMA wait windows via tile_wait_until schedule stamps.
"""
import numpy as np
import concourse.bass as bass
import concourse.tile as tile
from concourse import bacc, mybir, bass_utils
from concourse.bass import IndirectOffsetOnAxis

N = 4096
N_P, N_F = 64, 64
BIGPEN = -131072.0
TMAX = N // 4
N_CORES = 8

_cache = {}


def build_kernel(tmax: int, goal: int, debug: bool = False):
    op = mybir.AluOpType
    f32 = mybir.dt.float32
    u32 = mybir.dt.uint32
    nc = bacc.Bacc("TRN2", target_bir_lowering=False, debug=debug)

    wadj = nc.dram_tensor("wadj", (N, N), f32, kind="ExternalInput").ap()
    g0_in = nc.dram_tensor("g0", (N_P, N_F), f32, kind="ExternalInput").ap()
    open0_in = nc.dram_tensor("open0", (N_P, N_F), f32, kind="ExternalInput").ap()
    h_in = nc.dram_tensor("h", (N_P, N_F), f32, kind="ExternalInput").ap()
    hneg_in = nc.dram_tensor("hneg", (N_P, N_F), f32, kind="ExternalInput").ap()
    iota_in = nc.dram_tensor("iota", (N_P, N_F), f32, kind="ExternalInput").ap()
    iotaP_in = nc.dram_tensor("iotaP", (N_P, 1), f32, kind="ExternalInput").ap()
    iotaP32_in = nc.dram_tensor("iotaP32", (N_P, 1), f32, kind="ExternalInput").ap()
    iota128r_in = nc.dram_tensor("iota128r", (1, N_P), f32, kind="ExternalInput").ap()
    ones_in = nc.dram_tensor("ones", (1, N_P), f32, kind="ExternalInput").ap()
    ident_in = nc.dram_tensor("ident", (128, 128), f32, kind="ExternalInput").ap()

    hist_out = nc.dram_tensor("hist_out", (N_P, N_F), f32, kind="ExternalOutput").ap()
    par_out = nc.dram_tensor("par_out", (N_P, N_F), f32, kind="ExternalOutput").ap()
    tf_out = nc.dram_tensor("tf_out", (1, 1), f32, kind="ExternalOutput").ap()

    wadj_chunks = wadj.rearrange("r (a b) -> (r a) b", b=N_F)

    with tile.TileContext(nc) as tc:
        with tc.tile_pool(name="state", bufs=1) as sp, \
             tc.tile_pool(name="scratch", bufs=1) as pool, \
             tc.tile_pool(name="ps", bufs=1, space="PSUM") as psp, \
             tc.tile_pool(name="ps2", bufs=2, space="PSUM") as psp2:
            g = sp.tile([N_P, N_F], f32, tag="g")
            hist = sp.tile([N_P, N_F], f32, tag="hist")
            closed = sp.tile([N_P, N_F], f32, tag="closed")
            parents = sp.tile([N_P, N_F], f32, tag="parents")
            val = sp.tile([N_P, N_F], f32, tag="val")
            ndb = sp.tile([N_P, 1], f32, tag="ndb")
            tf = sp.tile([1, 1], f32, tag="tf")
            h_t = sp.tile([N_P, N_F], f32, tag="h_t")
            hneg_t = sp.tile([N_P, N_F], f32, tag="hneg_t")
            iota = sp.tile([N_P, N_F], f32, tag="iota")
            itP = sp.tile([N_P, 1], f32, tag="itP")
            itP32 = sp.tile([N_P, 1], f32, tag="itP32")
            i128r = sp.tile([1, N_P], f32, tag="i128r")
            ones_t = sp.tile([1, N_P], f32, tag="ones_t")
            ident_t = sp.tile([128, 128], f32, tag="ident_t")

            nc.sync.dma_start(g, g0_in)
            nc.sync.dma_start(closed, open0_in)
            nc.sync.dma_start(h_t, h_in)
            nc.sync.dma_start(hneg_t, hneg_in)
            nc.sync.dma_start(iota, iota_in)
            nc.sync.dma_start(itP, iotaP_in)
            nc.sync.dma_start(itP32, iotaP32_in)
            nc.sync.dma_start(i128r, iota128r_in)
            nc.sync.dma_start(ones_t, ones_in)
            nc.sync.dma_start(ident_t, ident_in)
            nc.vector.memset(hist, 0.0)
            nc.vector.memset(parents, float(goal))
            nc.vector.memset(ndb, 1.0)
            nc.vector.memset(tf, 0.0)

            # preamble: val = ((hist>=closed) * BIGPEN) - (g+h)
            e0 = pool.tile([N_P, N_F], f32, tag="e0")
            gh0 = pool.tile([N_P, N_F], f32, tag="gh0")
            nc.vector.tensor_tensor(out=e0, in0=hist, in1=closed, op=op.is_ge)
            nc.vector.tensor_tensor(out=gh0, in0=g, in1=h_t, op=op.add)
            nc.vector.scalar_tensor_tensor(out=val, in0=e0, scalar=BIGPEN, in1=gh0,
                                           op0=op.mult, op1=op.subtract)

            deferred = []  # (idx, idxm, t1, bc0) from previous step

            STEP_MS = 6.6 * 1e-3
            def stamp(t, off_us):
                return tc.tile_wait_until(t * STEP_MS + off_us * 1e-3)

            for t in range(tmax):
                # ---- phase 1: level-1 argmax ----
                m8 = pool.tile([N_P, 8], f32, tag="m8")
                mi8 = pool.tile([N_P, 8], u32, tag="mi8")
                stamp_ctx = stamp(t, 0.9); stamp_ctx.__enter__()
                nc.vector.max(out=m8, in_=val)
                nc.vector.max_index(out=mi8, in_max=m8, in_values=val)
                t_max = psp.tile([1, N_P], f32, tag="t_max")
                nc.tensor.transpose(t_max, m8[:, 0:1], ident_t[0:N_P, 0:N_P])
                gidxF = pool.tile([N_P, 1], f32, tag="gidxF")
                nc.vector.tensor_scalar(out=gidxF, in0=mi8[:, 0:1], scalar1=1.0,
                                        scalar2=itP32[:, 0:1], op0=op.mult, op1=op.add)

                # ---- phase 3: level-2 argmax + index select ----
                t_gidx = psp.tile([1, N_P], f32, tag="t_gidx")
                nc.tensor.transpose(t_gidx, gidxF, ident_t[0:N_P, 0:N_P])
                gmax8 = pool.tile([1, 8], f32, tag="gmax8")
                pstar8 = pool.tile([1, 8], u32, tag="pstar8")
                nc.vector.max(out=gmax8, in_=t_max)
                nc.vector.max_index(out=pstar8, in_max=gmax8, in_values=t_max)
                pstarF = pool.tile([1, 1], f32, tag="pstarF")
                nc.vector.tensor_copy(pstarF, pstar8[0:1, 0:1])
                junkr = pool.tile([1, N_P], f32, tag="junkr")
                indF = pool.tile([1, 1], f32, tag="indF")
                nc.vector.scalar_tensor_tensor(
                    out=junkr, in0=i128r, scalar=pstarF[0:1, 0:1], in1=t_gidx[0:1, :],
                    op0=op.is_equal, op1=op.mult, accum_out=indF)

                # ---- phase 4: broadcast + gather ----
                bc0 = psp2.tile([N_P, 1], f32, tag="bc0")
                nc.tensor.matmul(bc0, lhsT=ones_t, rhs=indF, start=True, stop=True)
                idxP = pool.tile([N_P, 1], u32, tag="idxP")
                nc.vector.tensor_scalar(out=idxP, in0=bc0[:, 0:1], scalar1=float(N_P),
                                        scalar2=itP[:, 0:1], op0=op.mult, op1=op.add)
                row_t = pool.tile([N_P, N_F], f32, tag="row_t")
                nc.gpsimd.indirect_dma_start(
                    out=row_t[:, :], out_offset=None, in_=wadj_chunks,
                    in_offset=IndirectOffsetOnAxis(ap=idxP[:, 0:1], axis=0))
                stamp_ctx.__exit__(None, None, None)

                # deferred g/closed/parents updates from the previous step at 3.2
                if deferred:
                    d_idx, d_idxm, d_t1, d_bc0 = deferred.pop()
                    with stamp(t, 3.2):
                        nc.vector.copy_predicated(g, d_idxm, d_t1)
                        nc.vector.tensor_tensor(out=closed, in0=closed, in1=d_idx, op=op.add)
                        nc.vector.copy_predicated(parents, d_idxm,
                                                  d_bc0[:, 0:1].to_broadcast([N_P, N_F]))

                # ---- phase 5: flight at 3.6 ----
                stamp_ctx = stamp(t, 3.6); stamp_ctx.__enter__()
                ohg = pool.tile([N_P, N_F], f32, tag="ohg")
                nc.vector.tensor_scalar(out=ohg, in0=iota, scalar1=bc0[:, 0:1],
                                        scalar2=ndb[:, 0:1], op0=op.is_equal, op1=op.mult)
                # val[ind] += BIGPEN  (masks the selected node out of the frontier)
                nc.vector.scalar_tensor_tensor(out=val, in0=ohg, scalar=BIGPEN, in1=val,
                                               op0=op.mult, op1=op.add)
                junkm = pool.tile([N_P, N_F], f32, tag="junkm")
                rs = pool.tile([N_P, 1], f32, tag="rs")
                nc.vector.scalar_tensor_tensor(
                    out=junkm, in0=ohg, scalar=1.0, in1=g,
                    op0=op.mult, op1=op.mult, accum_out=rs)
                nc.vector.tensor_tensor(out=hist, in0=hist, in1=ohg, op=op.max)
                avail = pool.tile([N_P, N_F], f32, tag="avail")
                nc.vector.tensor_scalar(out=avail, in0=closed, scalar1=0.0,
                                        scalar2=ndb[:, 0:1], op0=op.is_equal, op1=op.mult)
                nc.scalar.activation(tf, tf, mybir.ActivationFunctionType.Identity,
                                     bias=ndb[0:1, 0:1])
                nc.vector.tensor_scalar(out=ndb, in0=bc0[:, 0:1], scalar1=float(goal),
                                        scalar2=ndb[:, 0:1], op0=op.not_equal, op1=op.mult)
                t_rs = psp.tile([1, N_P], f32, tag="t_rs")
                nc.tensor.transpose(t_rs, rs, ident_t[0:N_P, 0:N_P])
                gind_s = pool.tile([1, 1], f32, tag="gind_s")
                nc.vector.reduce_sum(gind_s, t_rs[0:1, :], axis=mybir.AxisListType.X)
                gb = psp.tile([N_P, 1], f32, tag="gb")
                nc.tensor.matmul(gb, lhsT=ones_t, rhs=gind_s, start=True, stop=True)
                stamp_ctx.__exit__(None, None, None)

                # ---- phase 6: post (needs row_t) at next-step 0.0 ----
                stamp_ctx = stamp(t + 1, 0.0); stamp_ctx.__enter__()
                idx = pool.tile([N_P, N_F], f32, tag="idx")
                nc.vector.scalar_tensor_tensor(out=idx, in0=row_t, scalar=0.0, in1=avail,
                                               op0=op.not_equal, op1=op.mult)
                idxm = pool.tile([N_P, N_F], mybir.dt.uint8, tag="idxm")
                nc.vector.tensor_copy(idxm, idx)
                t1 = pool.tile([N_P, N_F], f32, tag="t1")
                nc.vector.tensor_scalar(out=t1, in0=row_t, scalar1=gb[:, 0:1],
                                        scalar2=None, op0=op.add)
                vneg = pool.tile([N_P, N_F], f32, tag="vneg")
                nc.vector.scalar_tensor_tensor(out=vneg, in0=t1, scalar=-1.0, in1=hneg_t,
                                               op0=op.mult, op1=op.add)
                nc.vector.copy_predicated(val, idxm, vneg)
                stamp_ctx.__exit__(None, None, None)

                deferred.append((idx, idxm, t1, bc0))

            # flush deferred updates of the last step
            d_idx, d_idxm, d_t1, d_bc0 = deferred.pop()
            nc.vector.copy_predicated(g, d_idxm, d_t1)
            nc.vector.tensor_tensor(out=closed, in0=closed, in1=d_idx, op=op.add)
            nc.vector.copy_predicated(parents, d_idxm,
                                      d_bc0[:, 0:1].to_broadcast([N_P, N_F]))

            nc.sync.dma_start(hist_out, hist)
            nc.sync.dma_start(par_out, parents)
            nc.sync.dma_start(tf_out, tf)

    nc.compile()
    return nc




def make_inputs(wadj_clean: np.ndarray, h: np.ndarray, start: int) -> dict:
    g0 = wadj_clean[start].reshape(N_P, N_F).astype(np.float32)
    open0 = np.zeros((N,), np.float32)
    open0[start] = 1.0
    h2 = h.reshape(N_P, N_F).astype(np.float32)
    return {
        "wadj": np.ascontiguousarray(wadj_clean, np.float32),
        "g0": g0,
        "open0": open0.reshape(N_P, N_F),
        "h": h2,
        "hneg": (-h2),
        "iota": np.arange(N, dtype=np.float32).reshape(N_P, N_F),
        "iotaP": np.arange(N_P, dtype=np.float32).reshape(N_P, 1),
        "iotaP32": (np.arange(N_P, dtype=np.float32) * N_F).reshape(N_P, 1),
        "iota128r": np.arange(N_P, dtype=np.float32).reshape(1, N_P),
        "ones": np.ones((1, N_P), np.float32),
        "ident": np.eye(128, dtype=np.float32),
    }




def backtrack(parents_f: np.ndarray, tf_val: float, goal: int, tmax: int) -> np.ndarray:
    parents_i = parents_f.reshape(-1).astype(np.int32)
    path = np.zeros((N,), np.int32)
    path[goal] = 1
    t_final = int(round(tf_val)) - 1
    loc = parents_i[goal]
    for i in range(tmax):
        if i < t_final:
            path[loc] = 1
            loc = parents_i[loc]
    return path


def kernel(start_index, goal_index, cost_maps, nodes, adj, weighted_adj):
    start = int(np.asarray(start_index))
    goal = int(np.asarray(goal_index))
    h = np.asarray(cost_maps, dtype=np.float32)
    wadj = np.asarray(weighted_adj, dtype=np.float32)

    wadj_clean = np.where(np.isinf(wadj), 0.0, wadj).astype(np.float32)
    np.fill_diagonal(wadj_clean, 0.0)

    key = (TMAX, goal)
    if key not in _cache:
        _cache[key] = build_kernel(TMAX, goal)
    nc = _cache[key]

    kin = make_inputs(wadj_clean, h, start)
    res = bass_utils.run_bass_kernel_spmd(
        nc, [kin] * N_CORES, core_ids=list(range(N_CORES)))
    r0 = res.results[0]
    hist = np.asarray(r0["hist_out"], dtype=np.float32).reshape(N)
    par = np.asarray(r0["par_out"], dtype=np.float32).reshape(N)
    tf = float(np.asarray(r0["tf_out"]).reshape(-1)[0])
    path = backtrack(par, tf, goal, TMAX)
    return hist, path.astype(np.int32)



# revision 3
# speedup vs baseline: 1.0385x; 1.0385x over previous
"""V2 Trainium2 Bass kernel for the differentiable A* forward pass.

Contract: kernel(**inputs) takes FULL unsharded inputs and returns
(histories, path_maps) matching reference().

Design (vs the 6.28ms baseline):
- State layout [32, 128], min-convention val = g + h (+BIG for untouched /
  selected nodes). The serial per-step chain is 7 DVE ops + 1 indirect DMA.
- DRAM "bundle" per node r: 128 cols of WHM2[r,j] = adj ? w[r,j]+h[j]-h[r]
  : BIG, plus a column carrying r itself. Gathering row ind gives candidate
  values vm1 = WHM2[ind] + gmin directly (gmin = val[ind] = g[ind]+h[ind]),
  so no explicit g state and no value-extraction chain is needed.
- Cross-partition argmin via DVE StreamTranspose of a packed [32,64] tile
  (per-partition min + equality-extracted argmin index), avoiding PE
  round-trips on the critical path.
- val[ind] masking via match_replace against the PE-broadcast gmin (window).
- hist is reconstructed host-side from a per-step log of selected nodes;
  path from parents + t_final (host backtrack), so the only device outputs
  are parents and the log.
"""
import numpy as np
import concourse.bass as bass
import concourse.tile as tile
from concourse import bacc, mybir, bass_utils
from concourse.bass import IndirectOffsetOnAxis

N = 4096
P, F = 32, 128
BIG = float(2 ** 20)
TMAX = N // 4
N_CORES = 1
CW = 128          # bundle chunk width (WHM2 only)
NCH = N * P       # chunk count

_cache = {}

# schedule (ms units for tile_wait_until)
PRE_MS = 8.0e-3
STEP_MS = 4.2e-3
B_OFF_MS = 2.7e-3
ROW_OFFSETS_1D = True   # [1,32] offset AP for the indirect gather


def build_kernel(tmax: int, goal: int, start: int):
    op = mybir.AluOpType
    f32 = mybir.dt.float32
    u32 = mybir.dt.uint32
    u8 = mybir.dt.uint8
    nc = bacc.Bacc("TRN2", target_bir_lowering=False)

    bundle = nc.dram_tensor("bundle", (NCH, CW), f32, kind="ExternalInput").ap()
    val0_in = nc.dram_tensor("val0", (P, F), f32, kind="ExternalInput").ap()
    nav0_in = nc.dram_tensor("nav0", (P, F), f32, kind="ExternalInput").ap()
    par0_in = nc.dram_tensor("par0", (P, F), f32, kind="ExternalInput").ap()
    iota_in = nc.dram_tensor("iota32", (P, F), f32, kind="ExternalInput").ap()
    ones_in = nc.dram_tensor("ones", (1, P), f32, kind="ExternalInput").ap()
    gm8_in = nc.dram_tensor("gm8", (1, 16), f32, kind="ExternalInput").ap()
    ndb0_in = nc.dram_tensor("ndb0", (P, 1), f32, kind="ExternalInput").ap()

    par_out = nc.dram_tensor("par_out", (P, F), f32, kind="ExternalOutput").ap()
    log_out = nc.dram_tensor("log_out", (1, tmax + 8), f32, kind="ExternalOutput").ap()

    with tile.TileContext(nc) as tc:
        with tc.tile_pool(name="state", bufs=1) as sp, \
             tc.tile_pool(name="scratch", bufs=1) as pool, \
             tc.tile_pool(name="ps", bufs=1, space="PSUM") as psp:
            val = sp.tile([P, F], f32, tag="val")
            navail = sp.tile([P, F], f32, tag="navail")
            parents = sp.tile([P, F], f32, tag="parents")
            iota32 = sp.tile([P, F], f32, tag="iota32")
            ones_t = sp.tile([1, P], f32, tag="ones_t")
            gm8 = sp.tile([1, 16], f32, tag="gm8")
            ndb32 = sp.tile([P, 1], f32, tag="ndb32")
            logt = sp.tile([1, tmax + 8], f32, tag="logt")
            pack = sp.tile([P, 64], f32, tag="pack")
            indU = sp.tile([1, 1], u32, tag="indU")
            rowA = sp.tile([P, CW], f32, tag="rowA")
            rowB = sp.tile([P, CW], f32, tag="rowB")
            rows = [rowA, rowB]
            gbcA = psp.tile([P, 16], f32, tag="gbcA")
            gbcB = psp.tile([P, 16], f32, tag="gbcB")
            gbcs = [gbcA, gbcB]

            nc.sync.dma_start(val, val0_in)
            nc.sync.dma_start(navail, nav0_in)
            nc.sync.dma_start(parents, par0_in)
            nc.sync.dma_start(iota32, iota_in)
            nc.sync.dma_start(ones_t, ones_in)
            nc.sync.dma_start(gm8, gm8_in)
            nc.sync.dma_start(ndb32, ndb0_in)
            nc.vector.memset(logt, 0.0)
            nc.vector.memset(pack, 0.0)

            # initial gmin broadcast (gm8 col0 = h[start]) and first gather
            nc.tensor.matmul(gbcs[1], lhsT=ones_t, rhs=gm8[0:1, 0:16],
                             start=True, stop=True)
            nc.sync.dma_start(rows[1][:, :],
                              bundle[start * P:(start + 1) * P, :])

            def stamp(t_abs_ms):
                return tc.tile_wait_until(t_abs_ms)

            for m in range(1, tmax + 1):
                row = rows[m % 2]
                nextrow = rows[(m + 1) % 2]
                gbc_old = gbcs[m % 2]
                gbc_new = gbcs[(m + 1) % 2]
                T = PRE_MS + (m - 1) * STEP_MS
                last = (m == tmax)

                # ---- A: critical chain ----
                ctx = stamp(T); ctx.__enter__()
                vm2 = pool.tile([P, F], f32, tag="vm2")
                nc.vector.scalar_tensor_tensor(
                    out=vm2, in0=row[:, 0:F], scalar=gbc_old[:, 0:1],
                    in1=navail, op0=op.add, op1=op.max)
                if not last:
                    nc.vector.tensor_tensor(
                        out=val, in0=val, in1=vm2, op=op.min)
                    nc.vector.tensor_reduce(
                        out=pack[:, 0:1], in_=val,
                        axis=mybir.AxisListType.X, op=op.min)
                    junk = pool.tile([P, F], f32, tag="junk")
                    nc.vector.scalar_tensor_tensor(
                        out=junk, in0=val, scalar=pack[:, 0:1], in1=iota32,
                        op0=op.is_equal, op1=op.mult, accum_out=pack[:, 32:33])
                    prow = pool.tile([P, 64], f32, tag="prow")
                    nc.vector.transpose(prow, pack)
                    nc.vector.tensor_reduce(
                        out=gm8[0:1, 0:1], in_=prow[0:1, 0:32],
                        axis=mybir.AxisListType.X, op=op.min)
                    junk2 = pool.tile([1, P], f32, tag="junk2")
                    nc.vector.scalar_tensor_tensor(
                        out=junk2, in0=prow[0:1, 0:32], scalar=gm8[0:1, 0:1],
                        in1=prow[0:1, 32:64], op0=op.is_equal, op1=op.mult,
                        accum_out=gm8[0:1, 8:9])
                    nc.vector.tensor_scalar(
                        out=indU, in0=gm8[0:1, 8:9], scalar1=float(N - 1),
                        scalar2=0.0, op0=op.min, op1=op.max)
                    regv = nc.sync.value_load(indU[0:1, 0:1])
                    b0 = bundle[0:P, :]
                    dyn = bass.AP(b0.tensor, regv * (P * CW),
                                  [list(x) for x in b0.ap])
                    nc.sync.dma_start(nextrow[:, :], dyn)
                    # broadcast new gmin + ind for window ops + next macro
                    nc.tensor.matmul(gbc_new, lhsT=ones_t, rhs=gm8[0:1, 0:16],
                                     start=True, stop=True)
                ctx.__exit__(None, None, None)

                # ---- B: window ops ----
                ctx = stamp(T + B_OFF_MS); ctx.__enter__()
                if not last:
                    nc.vector.match_replace(
                        out=val, in_to_replace=gbc_new[:, 0:8], in_values=val,
                        imm_value=BIG)
                idxg = pool.tile([P, F], f32, tag="idxg")
                nc.vector.tensor_scalar(
                    out=idxg, in0=vm2, scalar1=1e5,
                    scalar2=ndb32[:, 0:1], op0=op.is_le, op1=op.mult)
                if not last:
                    nc.vector.scalar_tensor_tensor(
                        out=navail, in0=idxg, scalar=2.0 * BIG, in1=navail,
                        op0=op.mult, op1=op.add)
                idxu8 = pool.tile([P, F], u8, tag="idxu8")
                nc.scalar.copy(idxu8, idxg)
                nc.vector.copy_predicated(
                    parents, idxu8, gbc_old[:, 8:9].to_broadcast([P, F]))
                # log (uses OLD ndb), then ndb update
                nc.vector.tensor_scalar(
                    out=logt[0:1, m:m + 1], in0=gbc_old[0:1, 8:9], scalar1=1.0,
                    scalar2=ndb32[0:1, 0:1], op0=op.add, op1=op.mult)
                if not last:
                    nc.vector.tensor_scalar(
                        out=ndb32, in0=gbc_old[:, 8:9], scalar1=float(goal),
                        scalar2=ndb32[:, 0:1], op0=op.not_equal, op1=op.mult)
                ctx.__exit__(None, None, None)

            nc.sync.dma_start(par_out, parents)
            nc.sync.dma_start(log_out, logt)

    nc.compile()
    return nc


def make_inputs(wadj_clean: np.ndarray, h: np.ndarray, start: int, goal: int) -> dict:
    adj = wadj_clean != 0.0
    WHM2 = np.where(adj, (wadj_clean + h[None, :]) - h[:, None], BIG)
    WHM2 = WHM2.astype(np.float32)
    bundle = WHM2.reshape(N, P, CW)

    val0 = np.full(N, BIG, np.float32)
    nav0 = np.full(N, -BIG, np.float32)
    nav0[start] = BIG
    par0 = np.full(N, float(goal), np.float32)
    gm8 = np.full((1, 16), -1e30, np.float32)
    gm8[0, 0] = h[start]
    gm8[0, 8] = float(start)
    gm8[0, 9:16] = 0.0
    return {
        "bundle": np.ascontiguousarray(bundle.reshape(NCH, CW)),
        "val0": val0.reshape(P, F),
        "nav0": nav0.reshape(P, F),
        "par0": par0.reshape(P, F),
        "iota32": np.arange(N, dtype=np.float32).reshape(P, F),
        "ones": np.ones((1, P), np.float32),
        "gm8": gm8,
        "ndb0": np.full((P, 1), 1.0 if start != goal else 0.0, np.float32),
    }


def postprocess(par: np.ndarray, log: np.ndarray, start: int, goal: int, tmax: int):
    # log[m] records ind_{m-1} (ind_0 = start lands in log[1])
    sel = []
    for m in range(1, tmax + 1):
        v = float(log[m])
        if v > 0.0:
            sel.append(int(round(v)) - 1)
    hist = np.zeros(N, np.float32)
    if sel:
        hist[np.array(sel, np.int64)] = 1.0
    t_final = min(len(sel) - 1, tmax - 1)

    parents_i = par.reshape(-1).astype(np.int32)
    path = np.zeros(N, np.int32)
    path[goal] = 1
    loc = parents_i[goal]
    for i in range(tmax):
        if i < t_final:
            path[loc] = 1
            loc = parents_i[loc]
    return hist, path


def kernel(start_index, goal_index, cost_maps, nodes, adj, weighted_adj):
    start = int(np.asarray(start_index))
    goal = int(np.asarray(goal_index))
    h = np.asarray(cost_maps, dtype=np.float32)
    wadj = np.asarray(weighted_adj, dtype=np.float32)

    wadj_clean = np.where(np.isinf(wadj), 0.0, wadj).astype(np.float32)
    np.fill_diagonal(wadj_clean, 0.0)

    key = (TMAX, goal, start)
    if key not in _cache:
        _cache[key] = build_kernel(TMAX, goal, start)
    nc = _cache[key]

    kin = make_inputs(wadj_clean, h, start, goal)
    res = bass_utils.run_bass_kernel_spmd(
        nc, [kin] * N_CORES, core_ids=list(range(N_CORES)))
    r0 = res.results[0]
    par = np.asarray(r0["par_out"], dtype=np.float32)
    log = np.asarray(r0["log_out"], dtype=np.float32).reshape(-1)
    hist, path = postprocess(par, log, start, goal, TMAX)
    return hist, path.astype(np.int32)
